# revision 48
# baseline (speedup 1.0000x reference)
"""Trainium2 Bass kernel for a masked-attention block (MAB).

Computation (per batch element):
    Q = X@Wq + bq ; K = Y@Wk + bk ; V = Y@Wv + bv
    logits = per-head Qh@Kh^T / 32, masked keys -> -inf, softmax over keys
    attn   = A @ Vh (concat heads)
    O1 = LN(Q + attn; g1,b1)
    O  = LN(O1 + relu(O1@Wo + bo); g2,b2)

Sharding: pure data-parallel, one batch element per NeuronCore (B=8 = 8 cores).

On-device dataflow is "feature-major": activations live in SBUF transposed
([model_dim -> 8x128 partitions, token -> free]) so every matmul chains with
no transposes.  Softmax denominators and LayerNorm stats are partition-dim
reductions done with stationary matmuls (which also broadcast the result
across partitions for free).

Precision/engines: projections and logits run bf16 into fp32 PSUM (same
1 cycle/row as fp32r, FWL weight loads, half the DMA).  The attention
weights (exp) and V are fp8e4, so the softmax denominator and A@V run as
DoubleRow matmuls at 2 MACs/cycle.  The key mask never touches the exp:
masked keys are excluded by zeroing V rows (per-partition ACT scale) and by
a 0/1 fp8 mask-column matrix standing in for all-ones in the denominator;
exp ACT ops therefore span two key-subtiles (the ACT engine, not the PE, is
the attention-phase pacer, so exp op count matters).  LayerNorm rsigma is
exp(-0.5*ln(var+eps)) and the ACT table list is steered so the whole kernel
uses the single Exp+Ln table: no mid-kernel table swaps.

Schedule: queries run in two 512-wide chunks.  Attention chunk 0 interleaves
Q-proj chunk-1 do-groups between heads; attention chunk 1 interleaves
O-proj chunk-0 groups; LayerNorm DVE work always overlaps the next PE
phase.  The PE stream stays dense end-to-end, so HAM stays at K=8/8.
"""

import numpy as np
import ml_dtypes
from contextlib import ExitStack

import concourse.bass as bass
import concourse.mybir as mybir
import concourse.tile as tile
from concourse import bacc
from concourse.bass_utils import run_bass_kernel_spmd

P = 128
NX = 1024
NY = 1024
DIM = 1024
H = 8
KO = DIM // P          # 8 partition sub-tiles of the model dim
QC = 512               # query chunk
NQC = NX // QC         # 2
F32 = mybir.dt.float32
BF16 = mybir.dt.bfloat16
F8 = mybir.dt.float8e4
DR = mybir.MatmulPerfMode.DoubleRow
AF = mybir.ActivationFunctionType
ALU = mybir.AluOpType
SCALE = 1.0 / 32.0     # 1/sqrt(DIM)
EPS = 1e-5
BF = ml_dtypes.bfloat16
F8NP = ml_dtypes.float8_e4m3
VNAMES = ("bq", "bk", "bo", "g1", "b1", "g2", "b2")
NVEC = len(VNAMES)

_TBL_PATCHED = False
_LDW_PATCHED = False
# walrus ships --enable-ldw-opt=false; our K/V-proj ng-pairs and the LN-stat
# chains reuse the same stationary back-to-back, and with DoubleRow matmuls
# at ~107ns the un-deduped LDWEIGHTS time is exposed, so turn it on.
ENABLE_LDW_OPT = False


def _patch_ldw_opt():
    global _LDW_PATCHED
    if _LDW_PATCHED or not ENABLE_LDW_OPT:
        return
    import concourse.bass_utils as _bu
    _orig = _bu.run_command

    def _run_command(argv, **kwargs):
        argv = ["--enable-ldw-opt=true" if a == "--enable-ldw-opt=false" else a
                for a in argv]
        return _orig(argv, **kwargs)

    _bu.run_command = _run_command
    _LDW_PATCHED = True


def _patch_act_tables():
    """Steer every activation in this kernel to the ONE table that contains
    all of Exp/Ln/Identity/Relu/Copy (natural_log_exp_and_others) -> a single
    table load, no mid-kernel swaps.  act_func_set_id is positional into
    act_info.json, so the list ORDER must not change; instead strip this
    kernel's functions from every other table's set so the chooser picks the
    combined table at its true index."""
    global _TBL_PATCHED
    if _TBL_PATCHED:
        return
    orig = bacc.get_activation_tables

    def steered(arch):
        tabs = orig(arch)
        pref = "natural_log_exp_and_others"
        mine = {AF.Exp, AF.Ln, AF.Identity, AF.Relu, AF.Copy}
        if pref in tabs and mine <= set(tabs[pref]):
            return {k: (v if k == pref else set(v) - mine)
                    for k, v in tabs.items()}
        return tabs

    bacc.get_activation_tables = steered
    _TBL_PATCHED = True


def _build():
    _patch_act_tables()
    _patch_ldw_opt()
    nc = bacc.Bacc("TRN2", target_bir_lowering=False, debug=False,
                   enable_asserts=False)

    # ---- DRAM I/O (per-core shapes) ----
    XTb = nc.dram_tensor("XTb", [DIM, NX], BF16, kind="ExternalInput").ap()
    YT8 = nc.dram_tensor("YT8", [DIM, NY], F8, kind="ExternalInput").ap()
    VPK = nc.dram_tensor("VPK", [P, KO, NVEC], F32, kind="ExternalInput").ap()
    M01 = nc.dram_tensor("M01", [P, KO], F32, kind="ExternalInput").ap()
    MM8 = nc.dram_tensor("MM8", [P, KO, P], F8, kind="ExternalInput").ap()
    Wqb = nc.dram_tensor("Wqb", [DIM, DIM], BF16, kind="ExternalInput").ap()
    Wk8 = nc.dram_tensor("Wk8", [DIM, DIM], F8, kind="ExternalInput").ap()
    Wv8 = nc.dram_tensor("Wv8", [DIM, DIM], F8, kind="ExternalInput").ap()
    Wob = nc.dram_tensor("Wob", [DIM, DIM], BF16, kind="ExternalInput").ap()
    BVB = nc.dram_tensor("bvb", [NY], BF16, kind="ExternalInput").ap()
    OT = nc.dram_tensor("OT", [DIM, NX], F32, kind="ExternalOutput").ap()

    xt3 = XTb.rearrange("(ko p) q -> p ko q", p=P)
    yt3 = YT8.rearrange("(ko p) q -> p ko q", p=P)
    wq3 = Wqb.rearrange("(ko p) d -> p ko d", p=P)
    wk3 = Wk8.rearrange("(ko p) d -> p ko d", p=P)
    wv3 = Wv8.rearrange("(ko p) d -> p ko d", p=P)
    wo3 = Wob.rearrange("(ko p) d -> p ko d", p=P)
    ot3 = OT.rearrange("(do p) q -> p do q", p=P)

    with tile.TileContext(nc) as tc:
        with ExitStack() as octx:
            const = octx.enter_context(tc.tile_pool(name="const", bufs=1))
            big = octx.enter_context(tc.tile_pool(name="big", bufs=1))
            xqp = octx.enter_context(tc.tile_pool(name="xq", bufs=1))
            wop = octx.enter_context(tc.tile_pool(name="wop", bufs=1))
            iop = tc.alloc_tile_pool(name="io1", bufs=1)

            # ---- constants (packed into 4 small DMAs) ----
            vpk = const.tile([P, KO, NVEC], F32, tag="vpk", name="vpk")
            m01_sb = const.tile([P, KO], F32, tag="m01", name="m01_sb")
            mm8 = const.tile([P, KO, P], F8, tag="mm8", name="mm8")
            bv_sb = const.tile([1, DIM], BF16, tag="v_bv", name="bv_sb")
            V = {name: i for i, name in enumerate(VNAMES)}

            def vec(name, do):
                i = V[name]
                return vpk[:, do, i:i + 1]

            ones_bf = const.tile([P, P], BF16, tag="onesbf", name="ones_bf")
            nc.vector.memset(ones_bf, 1.0)
            eps_sb = const.tile([P, 1], F32, tag="eps", name="eps_sb")
            nc.vector.memset(eps_sb, EPS)

            # ---- long-lived activation tiles ----
            ktm = big.tile([P, KO, NY], BF16, tag="ktm", name="ktm")
            vm = big.tile([P, KO, DIM], F8, tag="vm", name="vm")
            qtb = big.tile([P, KO, NX], BF16, tag="qtb", name="qtb")

            # ============ Phase 1: K, Q(c0), V projections ============
            # DMA plan: K-proj operands stream per-k on the sync queue (the
            # PE chases the arrivals); the two const DMAs slot in after the
            # first pair; the bulk later-needed tensors go as single
            # triggers on the ACT queue (parallel trigger issue, ~0.7us per
            # trigger on a queue is the real cost, not bandwidth).
            ytb = iop.tile([P, KO, NY], F8, tag="ytb", name="ytb")
            wkt = iop.tile([P, KO, DIM], F8, tag="wkt", name="wkt")
            wvt = iop.tile([P, KO, DIM], F8, tag="wvt", name="wvt")
            xtb = xqp.tile([P, KO, NX], BF16, tag="xtb", name="xtb")
            wqt = xqp.tile([P, KO, DIM], BF16, tag="wqt", name="wqt")
            wot = wop.tile([P, KO, DIM], BF16, tag="wot", name="wot")
            # two trigger queues in parallel: sync streams ytb per-k (K-proj
            # chases it), scalar leads with wkt (the first thing the PE
            # needs), then consts + bulk tensors
            # K-proj operands stream as 2-k slices -- exactly one DoubleRow
            # contraction step each -- on the two trigger queues in
            # parallel, so the first matmul starts as soon as slice 0 of
            # both lands and the stream stays ahead of the PE
            for kp in range(4):
                nc.scalar.dma_start(wkt[:, 2 * kp:2 * kp + 2, :],
                                    wk3[:, 2 * kp:2 * kp + 2, :])
                nc.sync.dma_start(ytb[:, 2 * kp:2 * kp + 2, :],
                                  yt3[:, 2 * kp:2 * kp + 2, :])
            # remaining triggers in PE consumption order: vpk (K copies,
            # ~20us), wqt/xtb (Q-proj c0, ~24us), then the V/O operands
            nc.scalar.dma_start(vpk, VPK)
            nc.scalar.dma_start(wvt, wv3)
            nc.scalar.dma_start(bv_sb, BVB.rearrange("(one n) -> one n", one=1))
            nc.scalar.dma_start(m01_sb, M01)
            nc.sync.dma_start(xtb, xt3)
            nc.scalar.dma_start(wqt, wq3)
            nc.sync.dma_start(wot, wo3)
            nc.scalar.dma_start(mm8, MM8)

            with tc.tile_pool(name="pp1", bufs=4, space="PSUM") as pp:
                # ---- K-proj: ktm[p,do,key] = sum_k Wk[k,d] Y^T[k,key]
                for grp in range(2):
                    pss = [pp.tile([P, 2, QC], F32, tag="ps",
                                   name=f"ps_k{grp}{i}") for i in range(4)]
                    for kp in range(4):
                        for i in range(4):
                            do = grp * 4 + i
                            for ng in range(2):
                                nc.tensor.matmul(
                                    pss[i][:, ng, :],
                                    lhsT=wkt[:, 2 * kp:2 * kp + 2,
                                             do * P:(do + 1) * P],
                                    rhs=ytb[:, 2 * kp:2 * kp + 2,
                                            ng * QC:(ng + 1) * QC],
                                    start=(kp == 0), stop=(kp == 3),
                                    perf_mode=DR)
                    for i in range(4):
                        do = grp * 4 + i
                        nc.scalar.activation(
                            ktm[:, do, :], pss[i], AF.Identity,
                            bias=vec("bk", do), scale=1.0)

                # ---- V-proj (natural layout; per-free bias via K=1 MM;
                #      masked key rows zeroed by the per-partition scale)
                for yo in range(KO):
                    ps = pp.tile([P, 2, QC], F32, tag="ps", name=f"ps_v{yo}")
                    for kp in range(4):
                        for ng in range(2):
                            nc.tensor.matmul(
                                ps[:, ng, :],
                                lhsT=ytb[:, 2 * kp:2 * kp + 2,
                                         yo * P:(yo + 1) * P],
                                rhs=wvt[:, 2 * kp:2 * kp + 2,
                                        ng * QC:(ng + 1) * QC],
                                start=(kp == 0), stop=False, perf_mode=DR)
                    for ng in range(2):
                        nc.tensor.matmul(
                            ps[:, ng, :], lhsT=ones_bf[0:1, :],
                            rhs=bv_sb[:, ng * QC:(ng + 1) * QC],
                            start=False, stop=True)
                    nc.scalar.activation(
                        vm[:, yo, :], ps, AF.Identity,
                        scale=m01_sb[:, yo:yo + 1])

                # ---- Q-proj chunk 0 (chunk 1 is interleaved into attn(0))
                for do in range(KO):
                    ps = pp.tile([P, 2, QC], F32, tag="ps", name=f"ps_q0{do}")
                    pq = ps[:, 0, :]
                    for k in range(KO):
                        nc.tensor.matmul(
                            pq, lhsT=wqt[:, k, do * P:(do + 1) * P],
                            rhs=xtb[:, k, 0:QC],
                            start=(k == 0), stop=(k == KO - 1))
                    nc.scalar.activation(
                        qtb[:, do, 0:QC], pq, AF.Identity,
                        bias=vec("bq", do), scale=1.0)

            # ============ Phase 2+3: per-query-chunk pipeline ============
            # io1 (ytb/wkt/wvt, 48KB/part) is dead after phase 1; release it
            # so the stage pools below reuse its address space.
            iop.release()
            stg = octx.enter_context(tc.tile_pool(name="stg", bufs=1))
            ep = octx.enter_context(tc.tile_pool(name="exp", bufs=3))
            rp = octx.enter_context(tc.tile_pool(name="rcp", bufs=2))
            sqp = octx.enter_context(tc.tile_pool(name="sq", bufs=1))
            stp = octx.enter_context(tc.tile_pool(name="st", bufs=8))
            outp = octx.enter_context(tc.tile_pool(name="out", bufs=4))
            lgp = octx.enter_context(tc.tile_pool(name="lgp", bufs=2, space="PSUM"))
            avp = octx.enter_context(tc.tile_pool(name="avp", bufs=2, space="PSUM"))
            rlp = octx.enter_context(tc.tile_pool(name="rlp", bufs=2, space="PSUM"))

            zts = [stg.tile([P, KO, QC], BF16, tag="zz", bufs=3, name=f"zt{c}")
                   for c in range(NQC)]
            z2ts = [stg.tile([P, KO, QC], BF16, tag="zz", bufs=3, name=f"z2t{c}")
                    for c in range(NQC)]
            o1ts = [stg.tile([P, KO, QC], BF16, tag="o1", bufs=2, name=f"o1t{c}")
                    for c in range(NQC)]

            def logits_head(c, h):
                # logitsT[key, q] = sum_d K^T_h[d, key] Q^T_h[d, q]; exp on
                # ACT over two key-subtiles at once, fp8 out (no mask here).
                qs = slice(c * QC, (c + 1) * QC)
                et = ep.tile([P, KO, QC], F8, tag="exp", name=f"et{c}_{h}")
                for kp in range(4):
                    pl = lgp.tile([P, 2, QC], F32, tag="lg", name=f"pl{c}{h}{kp}")
                    for j in range(2):
                        kt = 2 * kp + j
                        nc.tensor.matmul(
                            pl[:, j, :],
                            lhsT=ktm[:, h, kt * P:(kt + 1) * P],
                            rhs=qtb[:, h, qs], start=True, stop=True)
                    nc.scalar.activation(
                        et[:, 2 * kp:2 * kp + 2, :], pl, AF.Exp, scale=SCALE)
                return et

            def denom_av_head(c, h, et):
                qs = slice(c * QC, (c + 1) * QC)
                # DoubleRow fp8: contract adjacent key-subtile pairs at
                # 2 MACs/cycle.  The 0/1 mask matrix replaces all-ones in
                # the denominator; masked V rows are already zero.
                pr = rlp.tile([P, QC], F32, tag="rl", name=f"pr{c}{h}")
                for kp in range(4):
                    nc.tensor.matmul(
                        pr, lhsT=mm8[:, 2 * kp:2 * kp + 2, :],
                        rhs=et[:, 2 * kp:2 * kp + 2, :],
                        start=(kp == 0), stop=(kp == 3), perf_mode=DR)
                rc = rp.tile([P, QC], F32, tag="rc", name=f"rc{c}{h}")
                nc.vector.reciprocal_approx_fast(rc, pr)
                pa = avp.tile([P, QC], F32, tag="av", name=f"pa{c}{h}")
                for kp in range(4):
                    nc.tensor.matmul(
                        pa, lhsT=vm[:, 2 * kp:2 * kp + 2, h * P:(h + 1) * P],
                        rhs=et[:, 2 * kp:2 * kp + 2, :],
                        start=(kp == 0), stop=(kp == 3), perf_mode=DR)
                nc.vector.tensor_mul(zts[c][:, h, :], pa, rc)
                nc.vector.tensor_add(zts[c][:, h, :], zts[c][:, h, :],
                                     qtb[:, h, qs])

            def qproj1_group(do):
                ps = avp.tile([P, QC], F32, tag="av", name=f"ps_q1{do}")
                for k in range(KO):
                    nc.tensor.matmul(
                        ps, lhsT=wqt[:, k, do * P:(do + 1) * P],
                        rhs=xtb[:, k, QC:NX],
                        start=(k == 0), stop=(k == KO - 1))
                # copy+bias on DVE: the ACT engine paces the surrounding
                # attention chunk (exp ops), keep it off ACT here
                nc.vector.tensor_scalar(
                    qtb[:, do, QC:NX], ps,
                    scalar1=vec("bq", do), scalar2=0.0,
                    op0=ALU.add, op1=ALU.bypass)

            def oproj_group(c, no):
                # H^T[n, q] = sum_d Wo[d, n] O1^T[d, q]; z2 = o1 + relu(H+bo)
                # chunk 0 runs inside attention chunk 1 where ACT paces the
                # exps -> relu on DVE there; chunk 1 runs post-attention
                # where ACT is idle and DVE is the congested engine.
                ps = avp.tile([P, QC], F32, tag="av", name=f"ps_o{c}{no}")
                for k in range(KO):
                    nc.tensor.matmul(
                        ps, lhsT=wot[:, k, no * P:(no + 1) * P],
                        rhs=o1ts[c][:, k, :],
                        start=(k == 0), stop=(k == KO - 1))
                ht = sqp.tile([P, QC], BF16, tag="ht", bufs=3,
                              name=f"ht{c}{no}")
                if c == 0:
                    nc.vector.tensor_scalar(
                        ht, ps, scalar1=vec("bo", no), scalar2=0.0,
                        op0=ALU.add, op1=ALU.max)
                else:
                    nc.scalar.activation(ht, ps, AF.Relu,
                                         bias=vec("bo", no), scale=1.0)
                nc.vector.tensor_add(z2ts[c][:, no, :], ht,
                                     o1ts[c][:, no, :])

            def attn_chunk(c, filler):
                # filler(i) emits one PE work-group between heads to keep
                # the PE fed while ACT drains the exps.
                prev = None
                fi = 0
                for h in range(H):
                    et = logits_head(c, h)
                    if filler is not None:
                        filler(fi); fi += 1
                    if prev is not None:
                        denom_av_head(c, h - 1, prev)
                    prev = et
                denom_av_head(c, H - 1, prev)
                return fi

            def layernorm(c, tag, in_sb, gname, bname, emit_out,
                          s0=0, s1=QC):
                S = s1 - s0
                sl = slice(s0, s1)
                pmu = rlp.tile([P, QC], F32, tag="rl", name=f"pmu{tag}{c}{s0}")
                ps2 = rlp.tile([P, QC], F32, tag="rl", name=f"ps2{tag}{c}{s0}")
                pmu = pmu[:, 0:S]
                ps2 = ps2[:, 0:S]
                for do in range(KO):
                    nc.tensor.matmul(pmu, lhsT=ones_bf,
                                     rhs=in_sb[:, do, sl],
                                     start=(do == 0), stop=(do == KO - 1))
                sqs = []
                for do in range(KO):
                    sq = sqp.tile([P, QC], BF16, tag="sq", bufs=8,
                                  name=f"sq{tag}{c}{s0}{do}")[:, 0:S]
                    nc.vector.tensor_mul(sq, in_sb[:, do, sl], in_sb[:, do, sl])
                    sqs.append(sq)
                for do in range(KO):
                    nc.tensor.matmul(ps2, lhsT=ones_bf, rhs=sqs[do],
                                     start=(do == 0), stop=(do == KO - 1))
                mu = stp.tile([P, QC], F32, tag="st", name=f"mu{tag}{c}{s0}")[:, 0:S]
                nc.vector.tensor_scalar_mul(mu, pmu, 1.0 / DIM)
                msq = stp.tile([P, QC], F32, tag="st", name=f"msq{tag}{c}{s0}")[:, 0:S]
                nc.vector.tensor_mul(msq, mu, mu)
                var = stp.tile([P, QC], F32, tag="st", name=f"var{tag}{c}{s0}")[:, 0:S]
                nc.vector.scalar_tensor_tensor(
                    var, ps2, 1.0 / DIM, msq,
                    op0=ALU.mult, op1=ALU.subtract)
                lnv = stp.tile([P, QC], F32, tag="st", name=f"lnv{tag}{c}{s0}")[:, 0:S]
                nc.scalar.activation(lnv, var, AF.Ln, bias=eps_sb, scale=1.0)
                rsig = stp.tile([P, QC], F32, tag="st", name=f"rsig{tag}{c}{s0}")[:, 0:S]
                nc.scalar.activation(rsig, lnv, AF.Exp, scale=-0.5)
                mub = stp.tile([P, QC], BF16, tag="stb", bufs=2,
                               name=f"mub{tag}{c}{s0}")[:, 0:S]
                nc.vector.tensor_copy(mub, mu)
                rsb = stp.tile([P, QC], BF16, tag="stb", bufs=2,
                               name=f"rsb{tag}{c}{s0}")[:, 0:S]
                nc.vector.tensor_copy(rsb, rsig)
                for do in range(KO):
                    t = sqp.tile([P, QC], BF16, tag="t", bufs=3,
                                 name=f"t{tag}{c}{s0}{do}")[:, 0:S]
                    nc.vector.tensor_sub(t, in_sb[:, do, sl], mub)
                    nc.vector.tensor_mul(t, t, rsb)
                    emit_out(do, t, sl)

            def ln1_chunk(c):
                def emit_o1(do, t, sl):
                    nc.vector.tensor_scalar(
                        o1ts[c][:, do, sl], t,
                        scalar1=vec("g1", do), scalar2=vec("b1", do),
                        op0=ALU.mult, op1=ALU.add)
                layernorm(c, "a", zts[c], "g1", "b1", emit_o1)

            def ln2_chunk(c, s0=0, s1=QC):
                def emit_o2(do, t, sl):
                    S = sl.stop - sl.start
                    o = outp.tile([P, QC], F32, tag="o",
                                  name=f"o{c}{sl.start}{do}")[:, 0:S]
                    nc.scalar.activation(
                        o, t, AF.Identity,
                        bias=vec("b2", do), scale=vec("g2", do))
                    nc.sync.dma_start(
                        ot3[:, do, c * QC + sl.start:c * QC + sl.stop], o)
                layernorm(c, "b", z2ts[c], "g2", "b2", emit_o2, s0, s1)

            # attn(0) fills with Q-proj chunk-1 groups; attn(1) fills with
            # O-proj chunk-0 groups (o1t(0) is ready once LN1(0)'s DVE apply
            # drains, a couple of heads in).
            attn_chunk(0, qproj1_group)
            ln1_chunk(0)

            oq = []

            def fill1(i):
                if i >= 2:
                    oproj_group(0, i - 2)
                    oq.append(i - 2)
            attn_chunk(1, fill1)
            for no in range(len(oq), KO):
                oproj_group(0, no)
            ln1_chunk(1)
            ln2_chunk(0)
            for no in range(KO):
                oproj_group(1, no)
            # split the last LayerNorm so its latency chain (stats -> rsig ->
            # apply -> out DMA) drains in two shorter pipelined halves
            ln2_chunk(1, 0, QC // 2)
            ln2_chunk(1, QC // 2, QC)

    nc.compile()
    return nc


_CACHE = {}


def _get_nc():
    if "nc" not in _CACHE:
        _CACHE["nc"] = _build()
    return _CACHE["nc"]


def make_in_maps(X, Y, mask, Wq, bq, Wk, bk, Wv, bv, Wo, bo, g1, b1, g2, b2):
    fb = lambda a: np.ascontiguousarray(np.asarray(a, dtype=np.float32).astype(BF))
    f8 = lambda a: np.ascontiguousarray(np.asarray(a, dtype=np.float32).astype(F8NP))
    shared = {
        "Wqb": fb(Wq), "Wk8": f8(Wk), "Wv8": f8(Wv), "Wob": fb(Wo),
        "bvb": fb(bv),
    }
    svecs = {
        "bq": np.asarray(bq, np.float32), "bk": np.asarray(bk, np.float32),
        "bo": np.asarray(bo, np.float32), "g1": np.asarray(g1, np.float32),
        "b1": np.asarray(b1, np.float32), "g2": np.asarray(g2, np.float32),
        "b2": np.asarray(b2, np.float32),
    }
    vpk0 = np.zeros((P, KO, NVEC), np.float32)
    for i, name in enumerate(VNAMES):
        vpk0[:, :, i] = svecs[name].reshape(KO, P).T
    X = np.asarray(X, dtype=np.float32)
    Y = np.asarray(Y, dtype=np.float32)
    mask = np.asarray(mask)
    in_maps = []
    for b in range(8):
        m01 = np.where(mask[b], np.float32(0.0), np.float32(1.0))
        m01_pk = np.ascontiguousarray(m01.reshape(KO, P).T)      # [P, KO]
        mm8 = np.ascontiguousarray(
            np.broadcast_to(m01_pk[:, :, None], (P, KO, P))).astype(F8NP)
        in_maps.append({
            "XTb": np.ascontiguousarray(X[b].T.astype(BF)),
            "YT8": np.ascontiguousarray(Y[b].T.astype(F8NP)),
            "VPK": vpk0,
            "M01": m01_pk,
            "MM8": mm8,
            **shared,
        })
    return in_maps


def kernel(X, Y, mask, Wq, bq, Wk, bk, Wv, bv, Wo, bo, g1, b1, g2, b2,
           _trace=False):
    nc = _get_nc()
    in_maps = make_in_maps(X, Y, mask, Wq, bq, Wk, bk, Wv, bv, Wo, bo,
                           g1, b1, g2, b2)
    res = run_bass_kernel_spmd(nc, in_maps, core_ids=list(range(8)),
                               trace=_trace)
    out = np.stack([np.ascontiguousarray(res.results[b]["OT"].T)
                    for b in range(8)]).astype(np.float32)
    if _trace:
        return out, res
    return out
